# revision 45
# baseline (speedup 1.0000x reference)
"""Bass/Trainium2 kernel for nn_BayesConv2dMF (per-sample-weight 3x3 conv).

Contract: kernel(**inputs) takes FULL unsharded inputs
  input      [32, 128, 56, 56] f32
  eps        [32, 128, 128, 3, 3] f32
  weight_psi [128, 128, 3, 3] f32
  weight_mu  [128, 128, 3, 3] f32
and returns the FULL output [32, 128, 56, 56] f32.

Strategy: data-parallel over batch across 8 NeuronCores (4 images/core).
Host prep (layout/dtype only -- exp, eps*exp(psi)+mu and the conv all stay
on device): psi/mu are fed pre-transposed as [CI, K9, CO] so the device can
DMA them in per-tap-group slices (the natural [CO,CI,3,3] layout has taps
innermost, making group slices 6-byte-granular). psi ships as fp16 and mu
as bf16 -- mu is consumed in bf16 anyway, and fp16 psi keeps exp(psi) to
~0.2%, negligible vs the 2e-2 gate.

Per image on-core (loads pipelined two images ahead, weights one ahead):
  eps -> SBUF via SWDGE cast-DMA f32->bf16 (natural [CO, CI*9] layout)
  per tap: PE transpose of eps -> PSUM [CI, k, CO] (bf16; image 0 is
      pipelined per 3-tap group so conv can start after the first group)
  DVE: wT = epsT * exp(psiT) then wT += muT  -> [CI, K9, CO] bf16
  x   -> plain [CI, 56, 56] bf16 tile via SWDGE cast-DMA (fully contiguous
      per partition -> full-rate; no padding needed at all)
  conv: chunks of 7 output rows; taps outer so one weight load feeds the
      live chunks; 9 PSUM-accumulating matmuls per chunk (K=CI=128, bf16).
      Edge handling: tap (1,1) goes first with start=True over the full
      chunk; kw!=1 taps restrict output columns and kh!=1 taps restrict
      rows on the top/bottom chunks (out-of-range x reads are implicit
      zeros that are simply never accumulated) -- the stream is the exact
      valid-MAC minimum, 166*166 columns/image.
  PSUM -> SBUF bf16 (ScalarE) -> DRAM bf16 (SP HWDGE); host upcasts.
  The final image tapers to 1-chunk parts (7,7,4,3 rows) so the drain
  tail shrinks. A HAM warm-up burst keeps the PE clock ungated through
  the input ramp.
"""

import numpy as np

import concourse.tile as tile
from concourse import bacc, mybir
from concourse.bass_utils import run_bass_kernel_spmd
from concourse.masks import make_identity

B, CO, CI, KH, KW, H, W = 32, 128, 128, 3, 3, 56, 56
K9 = KH * KW
N_CORES = 8
BPC = B // N_CORES  # images per core
RB = 7  # output rows per PSUM chunk
NCHUNK = H // RB  # 8 chunks per image
F32 = mybir.dt.float32
F16 = mybir.dt.float16
BF16 = mybir.dt.bfloat16

# tap-group order: g1 (taps 3,4,5) first so tap 4 = (kh=1,kw=1) leads
GROUPS = [1, 2, 0]
# within-chunk tap order: group g1 first, tap (kh,1) first inside each group
TAP_ORDER = [4, 3, 5, 7, 6, 8, 1, 0, 2]

N_WARM = 52  # HAM warm-up matmuls (must fit in the pre-conv PE idle window)

NPSO = 5  # rolling PSUM chunk slots (PSUM is 8 banks: 5 + pswta + pswtb + warm)

# image-0 x row pieces (prefix loads so early conv parts can start)
X0_BOUNDS = [0, 15, 29, H]


def tap_ranges(k):
    """Output-column range and x-column range for tap k (W-edge handling)."""
    kh, kw = divmod(k, KW)
    if kw == 0:
        return kh, 1, W, 0, W - 1  # out cols 1..55 <- x cols 0..54
    if kw == 2:
        return kh, 0, W - 1, 1, W  # out cols 0..54 <- x cols 1..55
    return kh, 0, W, 0, W  # full


def emit(nc, tc, ctx, x_d, eps_d, psit_d, mut_d, out_d):
    const = ctx.enter_context(tc.tile_pool(name="const", bufs=1))
    wpool = ctx.enter_context(tc.tile_pool(name="wpool", bufs=2))
    opool = ctx.enter_context(tc.tile_pool(name="opool", bufs=2))
    psw = ctx.enter_context(tc.tile_pool(name="psw", bufs=1, space="PSUM"))
    pso = ctx.enter_context(tc.tile_pool(name="pso", bufs=1, space="PSUM"))

    ident = const.tile([128, 128], BF16)
    # memsets on DVE so the Pool engine can start eps/x SWDGE descriptor
    # generation immediately (only the affine-select needs gpsimd)
    nc.vector.memset(ident, 0.0)
    make_identity(nc, ident, nomemset=True)
    # warm-up operand: a memset-only scratch tile, so the HAM burst starts
    # ~1us before the identity's affine-select completes
    scratch = const.tile([128, 128], BF16)
    nc.vector.memset(scratch, 0.0)

    # HAM warm-up: dummy matmuls fill the pre-conv PE idle window so the
    # activity monitor releases the clock gate before the real stream.
    warm_ps = psw.tile([128, 64], F32, tag="warm", name="warm_ps", bufs=1)
    for _ in range(N_WARM):
        nc.tensor.matmul(warm_ps, scratch, scratch[:, :64], start=True, stop=True)

    # shared weights, host-pre-transposed to [CI, K9, CO] (psi fp16 and mu
    # bf16 from the host: mu is consumed in bf16 anyway, and fp16 psi keeps
    # exp(psi) to ~0.2% which is negligible downstream)
    psi_t = const.tile([CI, K9, CO], F16)
    exp_psi = const.tile([CI, K9, CO], BF16)
    mu_bf = const.tile([CI, K9, CO], BF16)
    for g in GROUPS:
        sl = slice(3 * g, 3 * g + 3)
        nc.sync.dma_start(psi_t[:, sl, :], psit_d[:, sl, :])
        nc.sync.dma_start(mu_bf[:, sl, :], mut_d[:, sl, :])
    for g in GROUPS:
        sl = slice(3 * g, 3 * g + 3)
        nc.scalar.activation(
            exp_psi[:, sl, :], psi_t[:, sl, :], mybir.ActivationFunctionType.Exp
        )

    NXP = 3
    xts = [const.tile([CI, H, W], BF16, name=f"xt{i}", tag=f"xt{i}") for i in range(NXP)]

    wTs = {}
    out_sbs = {}
    eps_ts = {}

    def prep_load(b):
        # cast-DMA eps and x for image b (issued ~2 images ahead so the
        # weight transposes never make a sequencer wait on them)
        eps_t = wpool.tile([CO, CI, K9], BF16, tag="eps", name=f"eps{b}")
        nc.gpsimd.dma_start(
            eps_t, eps_d[b].rearrange("co ci kh kw -> co ci (kh kw)")
        )
        eps_ts[b] = eps_t
        xt = xts[b % NXP]
        bounds = X0_BOUNDS if b == 0 else [0, H]
        for lo, hi in zip(bounds[:-1], bounds[1:]):
            nc.gpsimd.dma_start(xt[:, lo:hi, :], x_d[b][:, lo:hi, :])
        out_sbs[b] = opool.tile([CO, H, W], BF16, tag="osb", name=f"osb{b}")

    def prep_w(b):
        # per-sample weights: transpose eps taps, then wT = epsT*exp(psiT)+muT
        eps_t = eps_ts[b]
        wT = wpool.tile([CI, K9, CO], BF16, tag="wT", name=f"wT{b}")
        # two bank-aligned PSUM tiles ([CI,8,CO] bf16 = exactly one bank):
        # A holds group g1 (taps 3..5), B holds taps 0..2 and 6..8 — so g1's
        # mul only waits on its own 3 transposes, not all 9 (tile-granular
        # dependency tracking)
        ps_a = psw.tile([CI, 8, CO], BF16, tag="pswta", name=f"pswta{b}", bufs=1)
        ps_b = psw.tile([CI, 8, CO], BF16, tag="pswtb", name=f"pswtb{b}", bufs=1)

        def ptile(k):
            return ps_a[:, k - 3, :] if 3 <= k <= 5 else ps_b[:, k if k < 3 else k - 3, :]

        for g in GROUPS:
            for k in range(3 * g, 3 * g + 3):
                nc.tensor.transpose(ptile(k), eps_t[:, :, k], ident)
        # mul/add per group: g1 from tile A, g0/g2 from tile B halves
        srcs = {1: ps_a[:, 0:3, :], 0: ps_b[:, 0:3, :], 2: ps_b[:, 3:6, :]}
        for g in GROUPS:
            sl = slice(3 * g, 3 * g + 3)
            nc.vector.tensor_mul(wT[:, sl, :], srcs[g], exp_psi[:, sl, :])
            nc.vector.tensor_add(wT[:, sl, :], wT[:, sl, :], mu_bf[:, sl, :])
        wTs[b] = wT

    slot_counter = [0]

    def conv_part(b, r0, nch, rb=RB, last=False, store=True):
        xt = xts[b % NXP]
        wT = wTs[b]
        out_sb = out_sbs[b]
        rows = nch * rb
        pss = []
        for c in range(nch):
            s = slot_counter[0] % NPSO
            slot_counter[0] += 1
            ps = pso.tile([CO, RB, W], F32, tag=f"pso{s}", name=f"ps_{b}_{r0}_{c}")
            pss.append(ps)
        # taps outer: one weight load per tap feeds all live chunk matmuls
        for i, k in enumerate(TAP_ORDER):
            kh, olo, ohi, xlo, xhi = tap_ranges(k)
            for c in range(nch):
                arow = r0 + c * rb
                rlo = 1 if (arow == 0 and kh == 0) else 0
                rhi = rb - (1 if (arow + rb == H and kh == 2) else 0)
                xr = arow + rlo + kh - 1
                nc.tensor.matmul(
                    pss[c][:, rlo:rhi, olo:ohi],
                    wT[:, k, :],
                    xt[:, xr : xr + rhi - rlo, xlo:xhi],
                    start=(i == 0),
                    stop=(i == K9 - 1),
                )
        for c in range(nch):
            dst = out_sb[:, r0 + c * rb : r0 + (c + 1) * rb, :]
            # final chunk: evacuate on DVE so it does not queue behind the
            # previous part's evacuation on the Activation engine
            if last:
                nc.vector.tensor_copy(dst, pss[c][:, :rb, :])
            else:
                nc.scalar.copy(dst, pss[c][:, :rb, :])
        if store:
            nc.sync.dma_start(
                out_d[b][:, r0 : r0 + rows, :], out_sb[:, r0 : r0 + rows, :]
            )

    # software-pipelined emission: loads run two images ahead, weight
    # transposes one image ahead (with eps already resident), interleaved
    # between the conv parts of the current image.
    # The final image tapers to 1-chunk parts so the drain tail shrinks.
    prep_load(0)
    prep_w(0)
    # second HAM burst: keeps PE activity through the wT-wait gap before
    # the first conv matmul (real-HW clock-gate insurance)
    for _ in range(24):
        nc.tensor.matmul(warm_ps, scratch, scratch[:, :64], start=True, stop=True)
    prep_load(1)
    for b in range(BPC):
        if b == 0:
            conv_part(b, 0, 2)
            conv_part(b, 2 * RB, 2)
            prep_w(1)
            conv_part(b, 4 * RB, 2)
            prep_load(2)
            conv_part(b, 6 * RB, 2)
        elif b + 1 < BPC:
            conv_part(b, 0, 2)
            conv_part(b, 2 * RB, 2)
            prep_w(b + 1)
            conv_part(b, 4 * RB, 2)
            if b + 2 < BPC:
                prep_load(b + 2)
            conv_part(b, 6 * RB, 2)
        else:
            conv_part(b, 0, 2)
            conv_part(b, 2 * RB, 2)
            conv_part(b, 4 * RB, 2)
            conv_part(b, 6 * RB, 1)
            conv_part(b, 7 * RB, 1, rb=4)
            conv_part(b, 7 * RB + 4, 1, rb=3, last=True)


def build():
    from contextlib import ExitStack

    nc = bacc.Bacc("TRN2", target_bir_lowering=False, debug=False, num_devices=N_CORES)
    x_d = nc.dram_tensor("input", [BPC, CI, H, W], F32, kind="ExternalInput").ap()
    eps_d = nc.dram_tensor(
        "eps", [BPC, CO, CI, KH, KW], F32, kind="ExternalInput"
    ).ap()
    psit_d = nc.dram_tensor("psi_t", [CI, K9, CO], F16, kind="ExternalInput").ap()
    mut_d = nc.dram_tensor("mu_t", [CI, K9, CO], BF16, kind="ExternalInput").ap()
    out_d = nc.dram_tensor("out", [BPC, CO, H, W], BF16, kind="ExternalOutput").ap()

    with tile.TileContext(nc) as tc:
        with ExitStack() as ctx:
            emit(nc, tc, ctx, x_d, eps_d, psit_d, mut_d, out_d)
    nc.compile()
    return nc


_NC_CACHE = None


def kernel(input, eps, weight_psi, weight_mu, **run_kwargs):
    global _NC_CACHE
    if _NC_CACHE is None:
        _NC_CACHE = build()
    nc = _NC_CACHE
    # host layout prep of the replicated weights: [CO,CI,KH,KW] -> [CI,K9,CO]
    import ml_dtypes

    psi_t = np.ascontiguousarray(
        weight_psi.reshape(CO, CI, K9).transpose(1, 2, 0), dtype=np.float16
    )
    mu_t = np.ascontiguousarray(
        weight_mu.reshape(CO, CI, K9).transpose(1, 2, 0).astype(ml_dtypes.bfloat16)
    )
    in_maps = []
    for c in range(N_CORES):
        sl = slice(c * BPC, (c + 1) * BPC)
        in_maps.append(
            {
                "input": np.ascontiguousarray(input[sl], dtype=np.float32),
                "eps": np.ascontiguousarray(eps[sl], dtype=np.float32),
                "psi_t": psi_t,
                "mu_t": mu_t,
            }
        )
    res = run_bass_kernel_spmd(
        nc, in_maps, core_ids=list(range(N_CORES)), **run_kwargs
    )
    out = np.concatenate(
        [np.asarray(res.results[c]["out"]) for c in range(N_CORES)], axis=0
    ).astype(np.float32)
    kernel._last_results = res
    return out
